# revision 19
# baseline (speedup 1.0000x reference)
"""Causal multi-head attention on 8 TRN2 NeuronCores.

Problem: x[4, 2048, 768], 12 heads x d_head 64, causal softmax attention.

Sharding: core c handles batch b = c//2 and the 6-head group h0 = 6*(c%2).
Each core computes its partial output o_partial[2048, 768] = sum over its 6
heads of (softmax(QK^T/8) V) @ W_O.  The two cores sharing a batch are summed
on the host (part of unsharding), so the device graph needs no collectives.

Device layout (per core) avoids every on-chip transpose:
  - host passes x^T  [768, 2048]  (xt)
  - Q^T, K^T [384, 2048] = W^T @ x^T   (lhsT = W slice, rhs = x^T)
  - V        [2048, 384]  = x @ W_V    (lhsT = x^T slice, rhs = W_V)
  - scores^T blocks [128k, 512q] = (K^T slice).T @ Q^T slice  (per head, K=64)
  - exp on ACT (scale=1/8 folded in); causal mask = 0/1 multiply on diagonal
    blocks; V is stored augmented with a ones column per head so the single
    AV matmul produces both z^T rows (64) and the softmax denominators (row 64)
  - z^T normalized via DMA-broadcast reciprocal row, stored as ZT [384, 2048]
  - out = (ZT).T @ W_O   (lhsT = ZT slice, rhs = W_O)
All matmuls run in bf16 (1 cycle/column vs 4 for fp32; f32 PSUM accum).

v2 (pipelined): the three phases are software-pipelined into ONE emission
stream so no engine ever sits idle behind a phase barrier:
  - PSUM budget 2(proj)+4(scores)+2(AV accum) banks lets projection /
    attention / output-projection pools coexist.
  - Attention runs q-supertile-serial (2 live AV accumulators instead of 4).
  - QK/V/output-projection matmuls are emitted as small "units" dripped
    between attention steps (1 unit per score step), with explicit deadline
    flushes so QT/KT/VA column ranges exist before the steps that read them.
    This keeps the PE instruction stream dense (HAM stays at 2.4 GHz) and
    starts the ACT exp chain ~4us into the kernel instead of ~78us.
  - Diagonal-block exps for the two packed heads are fused into one ACT
    instruction via a [128, 2, 512-q0] access pattern (halves ACT call count
    on the diagonal).
  - A dummy exp right after the first DMA pre-loads the ACT function table
    (~2.7us) off the critical path.
"""

import sys

if "/opt/trn_rl_repo" not in sys.path:
    sys.path.insert(0, "/opt/trn_rl_repo")

import numpy as np
import ml_dtypes

BF16NP = ml_dtypes.bfloat16


def _ensure_ntff_hook():
    """The agent image's `antenv` lacks `axon_hooks`, which bass_utils needs
    for trace=True under axon. Recreate it via sys.modules injection using the
    boot helper's ctypes wrapper around libaxon_pjrt.so."""
    import types
    if "antenv.axon_hooks" in sys.modules:
        return
    try:
        from trn_agent_boot.trn_boot import _ntff_profile_via_ctypes
        hook = _ntff_profile_via_ctypes("/opt/axon/libaxon_pjrt.so")
    except Exception:
        hook = None
    m = types.ModuleType("antenv.axon_hooks")
    m._hook = hook
    m.get_axon_ntff_profile_hook = lambda: m._hook
    def _set(h):
        m._hook = h
    m.set_axon_ntff_profile_hook = _set
    sys.modules["antenv.axon_hooks"] = m


_ensure_ntff_hook()

import concourse.bass as bass
import concourse.tile as tile
from concourse import bacc, mybir
from concourse.bass_utils import run_bass_kernel_spmd

F32 = mybir.dt.float32
BF16 = mybir.dt.bfloat16
AF = mybir.ActivationFunctionType

D = 768          # d_model
S = 2048         # seq
E = 64           # d_head
NHC = 6          # heads per core
HE = NHC * E     # 384
KD = D // 128    # 6 k-chunks over d_model
NQS = S // 512   # 4 q-supertiles
NKB = S // 128   # 16 k-blocks
B = 4

LAST_EXEC_TIME_NS = None
_GRAPH_CACHE = {}


def _build_graph(qkv_bias: bool) -> bass.Bass:
    nc = bacc.Bacc("TRN2", target_bir_lowering=False)
    xt = nc.declare_dram_parameter("xt", [D, S], BF16, isOutput=False)
    wq = nc.declare_dram_parameter("wq", [D, HE], BF16, isOutput=False)
    wk = nc.declare_dram_parameter("wk", [D, HE], BF16, isOutput=False)
    wv = nc.declare_dram_parameter("wv", [D, HE], BF16, isOutput=False)
    wo = nc.declare_dram_parameter("wo", [HE, D], BF16, isOutput=False)
    mask = nc.declare_dram_parameter("mask", [128, 128], BF16, isOutput=False)
    if qkv_bias:
        bq = nc.declare_dram_parameter("bq", [HE, 1], F32, isOutput=False)
        bk = nc.declare_dram_parameter("bk", [HE, 1], F32, isOutput=False)
        bv = nc.declare_dram_parameter("bv", [1, HE], BF16, isOutput=False)
    out = nc.declare_dram_parameter("out", [S, D], BF16, isOutput=True)

    with tile.TileContext(nc) as tc:
        with tc.tile_pool(name="persist", bufs=1) as persist, \
             tc.tile_pool(name="psP", bufs=2, space="PSUM") as psP, \
             tc.tile_pool(name="psS", bufs=2, space="PSUM") as psS, \
             tc.tile_pool(name="psZ", bufs=1, space="PSUM") as psZ, \
             tc.tile_pool(name="workE", bufs=12) as workE, \
             tc.tile_pool(name="workZ", bufs=8) as workZ, \
             tc.tile_pool(name="work2", bufs=4) as work2, \
             tc.tile_pool(name="workO", bufs=1) as workO, \
             tc.tile_pool(name="dramP", bufs=4, space="DRAM") as dramP:

            # ---------------- persistent tiles ----------------
            QT = [persist.tile([128, S], BF16, tag=f"qt{m}", name=f"qt{m}") for m in range(3)]
            # K^T per head, zero-padded to the full 128 partitions: head
            # h = 2m+par keeps its 64 K-dims in rows par*64..par*64+63 and
            # zeros elsewhere.  A score matmul is then a full-128-row matmul
            # (rhs = the shared QT tile; the other head's rows meet zero
            # weights), which the PE pipelines at N/2.4 with LDWEIGHTS hidden
            # -- 64-row-group matmuls pay the ~106ns weight load serially.
            KZ = [persist.tile([128, S], BF16, tag=f"kz{h}", name=f"kz{h}") for h in range(6)]
            ZT = [persist.tile([128, S], BF16, tag=f"zt{m}", name=f"zt{m}") for m in range(3)]
            VA = [persist.tile([128, NHC * 65], BF16, tag=f"va{s}", name=f"va{s}") for s in range(16)]
            WO = [persist.tile([128, D], BF16, tag=f"wo{m}", name=f"wo{m}") for m in range(3)]
            MSK = persist.tile([128, 128], BF16, tag="mask", name="mask_sb")
            XT = [persist.tile([128, S], BF16, tag=f"xt{k}", name=f"xt{k}") for k in range(KD)]
            WQs = [persist.tile([128, HE], BF16, tag=f"wq{k}", name=f"wq{k}") for k in range(KD)]
            OB = [workO.tile([128, D], BF16, tag=f"ob{i}", name=f"ob{i}") for i in range(4)]
            WKs = [persist.tile([128, HE], BF16, tag=f"wk{k}", name=f"wk{k}") for k in range(KD)]
            WVs = [persist.tile([128, HE], BF16, tag=f"wv{k}", name=f"wv{k}") for k in range(KD)]

            # loads ordered so the prologue's inputs land first; xt comes in
            # column blocks because the first projection unit contracts over
            # ALL six 128-row chunks but only the first 512 columns
            nc.sync.dma_start(out=MSK[:], in_=mask[:])
            for k in range(KD):
                nc.sync.dma_start(out=WKs[k][:], in_=wk[k * 128:(k + 1) * 128, :])
                nc.sync.dma_start(out=WQs[k][:], in_=wq[k * 128:(k + 1) * 128, :])
            for n in range(4):
                for k in range(KD):
                    nc.sync.dma_start(
                        out=XT[k][:, n * 512:(n + 1) * 512],
                        in_=xt[k * 128:(k + 1) * 128, n * 512:(n + 1) * 512])
                if n == 0:
                    for k in range(KD):
                        nc.sync.dma_start(out=WVs[k][:], in_=wv[k * 128:(k + 1) * 128, :])
            for m in range(3):
                nc.sync.dma_start(out=WO[m][:], in_=wo[m * 128:(m + 1) * 128, :])
            ONES = persist.tile([1, 128], BF16, tag="ones", name="ones_sb")
            nc.vector.memset(ONES[:], 1.0)
            ONESB = persist.tile([128, 64], BF16, tag="onesb", name="onesb_sb")
            nc.vector.memset(ONESB[:], 1.0)
            if qkv_bias:
                BQ = persist.tile([128, 3], F32, tag="bq", name="bq_sb")
                BK = persist.tile([128, 3], F32, tag="bk", name="bk_sb")
                BV = persist.tile([1, HE], BF16, tag="bv", name="bv_sb")
                for m in range(3):
                    nc.sync.dma_start(out=BQ[:, m:m + 1], in_=bq[m * 128:(m + 1) * 128, :])
                    nc.sync.dma_start(out=BK[:, m:m + 1], in_=bk[m * 128:(m + 1) * 128, :])
                nc.sync.dma_start(out=BV[:], in_=bv[:])

            # warm the ACT exp table off the critical path (~2.7us table load)
            WRM = persist.tile([1, 128], BF16, tag="wrm", name="wrm_sb")
            nc.scalar.activation(WRM[:], MSK[0:1, :], AF.Exp, scale=0.125)

            # zero the pad halves of KZ once (GpSimd is otherwise idle)
            for h in range(6):
                zr0 = (1 - (h % 2)) * 64
                nc.any.memset(KZ[h][zr0:zr0 + 64, :], 0.0)

            # ---------------- projection / output units ----------------
            def qk_unit(is_q, m, n):
                def fn():
                    Wt = WQs if is_q else WKs
                    ps = psP.tile([128, 512], F32, tag="psp", name="ps_p")
                    for k in range(KD):
                        nc.tensor.matmul(
                            ps[:],
                            Wt[k][:, m * 128:(m + 1) * 128],
                            XT[k][:, n * 512:(n + 1) * 512],
                            start=(k == 0), stop=(k == KD - 1))
                    ncol = slice(n * 512, (n + 1) * 512)
                    if is_q:
                        dst = QT[m][:, ncol]
                        if qkv_bias:
                            nc.scalar.activation(dst, ps[:], AF.Copy,
                                                 bias=BQ[:, m:m + 1])
                        else:
                            nc.vector.tensor_copy(dst, ps[:])
                    else:
                        # split the two heads into their padded KZ tiles
                        # (partition ranges stay aligned: 0-63 -> 0-63)
                        for par in (0, 1):
                            rows = slice(par * 64, par * 64 + 64)
                            dst = KZ[2 * m + par][rows, ncol]
                            if qkv_bias:
                                nc.scalar.activation(
                                    dst, ps[rows, :], AF.Copy,
                                    bias=BK[rows, m:m + 1])
                            else:
                                nc.vector.tensor_copy(dst, ps[rows, :])
                return fn

            def v_unit(sc):
                def fn():
                    va_v = VA[sc][:].rearrange("p (h c) -> p h c", c=65)
                    nc.vector.memset(va_v[:, :, 64:65], 1.0)
                    ps = psP.tile([128, HE], F32, tag="psp", name="ps_p")
                    for k in range(KD):
                        nc.tensor.matmul(
                            ps[:],
                            XT[k][:, sc * 128:(sc + 1) * 128],
                            WVs[k][:],
                            start=(k == 0), stop=False if qkv_bias else (k == KD - 1))
                    if qkv_bias:
                        nc.tensor.matmul(
                            ps[:], ONES[:], BV[:],
                            start=False, stop=True)
                    nc.vector.tensor_copy(
                        va_v[:, :, 0:64],
                        ps[:].rearrange("p (h c) -> p h c", c=64))
                return fn

            def out_unit(mc, half):
                # half 0 computes + stages both halves' left part; half 1
                # finishes the row block and issues ONE [128, 768] bf16 DMA
                def fn():
                    po = psP.tile([128, HE], F32, tag="psp", name="ps_p")
                    n0 = half * HE
                    for k in range(3):
                        nc.tensor.matmul(
                            po[:],
                            ZT[k][:, mc * 128:(mc + 1) * 128],
                            WO[k][:, n0:n0 + HE],
                            start=(k == 0), stop=(k == 2))
                    ob = OB[mc % 4]
                    nc.vector.tensor_copy(ob[:, n0:n0 + HE], po[:])
                    if half == 1:
                        nc.sync.dma_start(
                            out=out[mc * 128:(mc + 1) * 128, :],
                            in_=ob[:])
                return fn

            units = {}
            order = []      # names, in preferred drip order
            emitted = set()

            def add_unit(name, fn):
                units[name] = fn
                order.append(name)

            def emit_unit(name):
                if name not in emitted:
                    emitted.add(name)
                    units[name]()

            drip_pos = [0]

            def pump_units(k):
                while k > 0 and drip_pos[0] < len(order):
                    nm = order[drip_pos[0]]
                    drip_pos[0] += 1
                    if nm not in emitted:
                        emit_unit(nm)
                        k -= 1

            # prologue: first column block of Q/K for head pair 0
            add_unit("K0n0", qk_unit(False, 0, 0))
            add_unit("Q0n0", qk_unit(True, 0, 0))
            emit_unit("K0n0")
            emit_unit("Q0n0")
            # drip order: V early (AV needs it), next head-pair's Q/K before
            # its attention begins, column block n before supertile t=n.
            drip = [("V0", v_unit(0)), ("V1", v_unit(1)),
                    ("K0n1", qk_unit(False, 0, 1)), ("Q0n1", qk_unit(True, 0, 1)),
                    ("V2", v_unit(2)), ("V3", v_unit(3)),
                    ("V4", v_unit(4)), ("V5", v_unit(5)),
                    ("K0n2", qk_unit(False, 0, 2)), ("Q0n2", qk_unit(True, 0, 2)),
                    ("V6", v_unit(6)), ("V7", v_unit(7)),
                    ("V8", v_unit(8)), ("V9", v_unit(9)),
                    ("K0n3", qk_unit(False, 0, 3)), ("Q0n3", qk_unit(True, 0, 3)),
                    ("V10", v_unit(10)), ("V11", v_unit(11)),
                    ("K1n0", qk_unit(False, 1, 0)), ("Q1n0", qk_unit(True, 1, 0)),
                    ("V12", v_unit(12)), ("V13", v_unit(13)),
                    ("V14", v_unit(14)), ("V15", v_unit(15)),
                    ("K1n1", qk_unit(False, 1, 1)), ("Q1n1", qk_unit(True, 1, 1)),
                    ("K1n2", qk_unit(False, 1, 2)), ("Q1n2", qk_unit(True, 1, 2)),
                    ("K1n3", qk_unit(False, 1, 3)), ("Q1n3", qk_unit(True, 1, 3)),
                    ("K2n0", qk_unit(False, 2, 0)), ("Q2n0", qk_unit(True, 2, 0)),
                    ("K2n1", qk_unit(False, 2, 1)), ("Q2n1", qk_unit(True, 2, 1)),
                    ("K2n2", qk_unit(False, 2, 2)), ("Q2n2", qk_unit(True, 2, 2)),
                    ("K2n3", qk_unit(False, 2, 3)), ("Q2n3", qk_unit(True, 2, 3))]
            for nm, fn in drip:
                add_unit(nm, fn)

            # ---------------- attention ----------------
            pending = []   # deferred normalize closures (keep PE fed)
            av_q = []      # aged AV work: (av_fn, norm_fn | None)
            AV_LAG = 10    # items of et ageing

            def drain_pending(upto):
                while len(pending) > upto:
                    pending.pop(0)()

            def pump_avs(lag):
                while len(av_q) > lag:
                    av_fn, norm_fn = av_q.pop(0)
                    av_fn()
                    if norm_fn is not None:
                        norm_fn()
                        drain_pending(1)

            out_ready = []  # out-proj units unlocked by finished hp2 normalizes

            def make_attention(hp):
                psz = {}       # key par -> psum tile for current supertile
                norm_done = [0, 0, 0, 0]

                def emit_normalize(par, t):
                    ho = par * 64
                    # drain psz out of PSUM fast (frees the bank)
                    zraw = workZ.tile([65, 512], BF16, tag="zraw", name="zraw")
                    nc.vector.tensor_copy(zraw[:], psz[par][:])

                    # broadcast the denominator row across 64 partitions on
                    # the PE (ones[1,64]^T @ row[1,512]), then a 64-lane
                    # reciprocal -- no DRAM bounce, no DMA latency.
                    psB = psP.tile([64, 512], F32, tag="psp", name="ps_bc")
                    nc.tensor.matmul(psB[:], ONESB[64:65, :], zraw[64:65, :],
                                     start=True, stop=True)
                    bc = work2.tile([64, 512], BF16, tag="bc", name="bc")
                    with nc.allow_low_precision(reason="softmax recip bf16"):
                        nc.vector.reciprocal(bc[:], psB[:])

                    def part2():
                        # final scale, deferred so the recip chain is hidden
                        nc.vector.tensor_mul(
                            ZT[hp][ho:ho + 64, t * 512:(t + 1) * 512],
                            zraw[0:64, :], bc[:])
                        norm_done[t] += 1
                        if hp == 2 and norm_done[t] == 2:
                            for mc in range(4 * t, 4 * t + 4):
                                for half in (0, 1):
                                    nm = f"O{mc}h{half}"
                                    add_unit(nm, out_unit(mc, half))
                    pending.append(part2)

                def emit_step(t, j):
                    r = j - 4 * t  # >= 0 only on the diagonal
                    q0 = 128 * r if r >= 0 else 0
                    # both heads' scores in ONE 2-bank psum tile; each score
                    # matmul uses the full-128-row padded KZ weights so the
                    # weight load pipelines behind the previous matmul
                    pss = psS.tile([128, 1024], F32, tag="pss", name="pss")
                    for par in (0, 1):
                        nc.tensor.matmul(
                            pss[:, par * 512 + q0:par * 512 + 512],
                            KZ[2 * hp + par][:, j * 128:(j + 1) * 128],
                            QT[hp][:, t * 512 + q0:(t + 1) * 512],
                            start=True, stop=True)
                    et = workE.tile([128, 1024], BF16, tag="et", name="et")
                    if r >= 0:
                        # diagonal: one fused exp over both heads' valid
                        # [q0:512] spans via a [128, 2, 512-q0] AP
                        et3 = et[:].rearrange("p (two q) -> p two q", two=2)
                        ps3 = pss[:].rearrange("p (two q) -> p two q", two=2)
                        nc.scalar.activation(
                            et3[:, :, q0:512], ps3[:, :, q0:512],
                            AF.Exp, scale=0.125)
                        for par in (0, 1):
                            nc.vector.tensor_mul(
                                et[:, par * 512 + q0:par * 512 + q0 + 128],
                                et[:, par * 512 + q0:par * 512 + q0 + 128],
                                MSK[:])
                    else:
                        nc.scalar.activation(et[:], pss[:],
                                             AF.Exp, scale=0.125)
                    for par in (0, 1):
                        def av_fn(par=par, et=et, j=j, t=t, q0=q0):
                            if j == 0:
                                psz[par] = psZ.tile(
                                    [65, 512], F32,
                                    tag=f"psz{par}", name=f"psz{par}")
                            h = 2 * hp + par
                            nc.tensor.matmul(
                                psz[par][:, q0:512],
                                VA[j][:, h * 65:(h + 1) * 65],
                                et[:, par * 512 + q0:par * 512 + 512],
                                start=(j == 0), stop=(j == 4 * t + 3))
                        norm_fn = (
                            lambda par=par, t=t, f=emit_normalize:
                            f(par, t)) if j == 4 * t + 3 else None
                        av_q.append((av_fn, norm_fn))

                return emit_step

            step_no = [0]
            for hp in range(3):
                emit_step = make_attention(hp)
                # hp2 processes the smallest supertile LAST so the final
                # normalize + output-projection tail is as short as possible
                t_order = [1, 2, 3, 0] if hp == 2 else list(range(NQS))
                for t in t_order:
                    # deadline flush: Q/K column blocks this supertile reads
                    for tp in range(t + 1):
                        emit_unit(f"K{hp}n{tp}")
                        emit_unit(f"Q{hp}n{tp}")
                    if hp == 0:
                        for sc in range(min(4 * t + 4, 16)):
                            emit_unit(f"V{sc}")
                    for j in range(4 * t + 4):
                        emit_step(t, j)
                        pump_avs(AV_LAG)
                        # drip ~0.5 units/step: spreads projection work across
                        # all head-pairs (deadline flushes cover stragglers)
                        step_no[0] += 1
                        if hp == 2 or step_no[0] % 2 == 0:
                            pump_units(1)
            pump_avs(0)
            drain_pending(0)
            pump_units(len(order))
    nc.compile()
    return nc


def _build_mask() -> np.ndarray:
    # triangle for the strict-diagonal 128x128 strip: 1.0 iff q_local >= k_local
    kl = np.arange(128)[:, None]
    ql = np.arange(128)[None, :]
    return (ql >= kl).astype(np.float32)


def kernel(**inputs) -> np.ndarray:
    global LAST_EXEC_TIME_NS
    x = np.asarray(inputs["normalized_resid_pre"], dtype=np.float32)
    W_Q = np.asarray(inputs["W_Q"], dtype=np.float32)
    W_K = np.asarray(inputs["W_K"], dtype=np.float32)
    W_V = np.asarray(inputs["W_V"], dtype=np.float32)
    W_O = np.asarray(inputs["W_O"], dtype=np.float32)
    b_Q = np.asarray(inputs["b_Q"], dtype=np.float32)
    b_K = np.asarray(inputs["b_K"], dtype=np.float32)
    b_V = np.asarray(inputs["b_V"], dtype=np.float32)
    b_O = np.asarray(inputs["b_O"], dtype=np.float32)

    qkv_bias = bool(b_Q.any() or b_K.any() or b_V.any())
    key = qkv_bias
    if key not in _GRAPH_CACHE:
        _GRAPH_CACHE[key] = _build_graph(qkv_bias)
    nc = _GRAPH_CACHE[key]

    mask = _build_mask()
    in_maps = []
    for c in range(8):
        b, h0 = c // 2, NHC * (c % 2)
        im = {
            "xt": np.ascontiguousarray(x[b].T).astype(BF16NP),
            "wq": np.ascontiguousarray(
                W_Q[h0:h0 + NHC].transpose(1, 0, 2).reshape(D, HE)).astype(BF16NP),
            "wk": np.ascontiguousarray(
                W_K[h0:h0 + NHC].transpose(1, 0, 2).reshape(D, HE)).astype(BF16NP),
            "wv": np.ascontiguousarray(
                W_V[h0:h0 + NHC].transpose(1, 0, 2).reshape(D, HE)).astype(BF16NP),
            "wo": np.ascontiguousarray(W_O[h0:h0 + NHC].reshape(HE, D)).astype(BF16NP),
            "mask": mask.astype(BF16NP),
        }
        if qkv_bias:
            im["bq"] = np.ascontiguousarray(b_Q[h0:h0 + NHC].reshape(HE, 1))
            im["bk"] = np.ascontiguousarray(b_K[h0:h0 + NHC].reshape(HE, 1))
            im["bv"] = np.ascontiguousarray(b_V[h0:h0 + NHC].reshape(1, HE)).astype(BF16NP)
        in_maps.append(im)

    import os
    trace = bool(os.environ.get("KERNEL_TRACE"))
    res = run_bass_kernel_spmd(nc, in_maps, core_ids=list(range(8)), trace=trace)
    LAST_EXEC_TIME_NS = res.exec_time_ns
    results = res.results

    out = np.empty((B, S, D), dtype=np.float32)
    for b in range(B):
        out[b] = (np.asarray(results[2 * b]["out"], dtype=np.float32) +
                  np.asarray(results[2 * b + 1]["out"], dtype=np.float32))
    if b_O.any():
        out += b_O
    return out
